# revision 66
# baseline (speedup 1.0000x reference)
"""Trainium2 Bass kernel for nn_MultiHeadAttention_75548474736720.

Linear-attention-style MHA with two causal prefix-sum bilinear forms,
evaluated with a chunked (linear-attention) reformulation (see v1 notes):
  A1 = elu(qh ph^T) + 1;  U[t,j] = sum_{s<=t} Sq[t,s] A1[s,j]
  W' = exp(U/(t+1)), den = sum_j W'
  out2[t,d] = (1/((t+1) den[t])) sum_{s<=t} (W'[t].A1[s]) vh[s,d]

v2: fp8 score path.  A1 and W' are stored fp8e4m3 (weight-side tensors
whose noise averages out over the j=1024 contractions).  The U strips and
the M (kh (x) A1 running state) updates run as fp8 DoubleRow matmuls
(2 s-blocks contracted per instruction).  A1^T / W'^T are produced by
uint16-pair-viewed DMA crossbar transposes of the fp8 tensors, giving a
j-pair-interleaved layout [jp, t|s, jlo]; the D_ii and W'@N contractions
address it with stride-2-byte APs, split into 4 j-pieces x even/odd.
Value-side tensors (vh, out2, projections) stay bf16; mixed-dtype
matmuls (fp8 lhsT x bf16 rhs) cover the boundaries.  M accumulates in
SBUF bf16 (DR delta + DVE add), freeing its 2 PSUM banks.

v4: the final projection streams per 128-row block with one output DMA
each (8-deep buffer ring), so the 2 MB output store pipelines with the
pipeline drain instead of trailing it (-2.8 us).

Sharding: 8 cores = (batch b in 0..1) x (head-group hg in 0..3, 4 heads
each); each core returns a partial [S, Dm] bf16 output summed on host.
"""

import sys

sys.path.insert(0, "/opt/trn_rl_repo")

import ml_dtypes
import numpy as np

import concourse.bass as bass  # noqa: F401  (registers AP machinery)
import concourse.mybir as mybir
from concourse import bacc
from concourse.tile import TileContext
from concourse.bass_utils import run_bass_kernel_spmd

F32 = mybir.dt.float32
BF16 = mybir.dt.bfloat16
FP8 = mybir.dt.float8e4
FP8E5 = mybir.dt.float8e5
U16 = mybir.dt.uint16
ACTF = mybir.ActivationFunctionType
ALU = mybir.AluOpType
DRMODE = mybir.MatmulPerfMode.DoubleRow
NPBF = ml_dtypes.bfloat16

B, S, DM, H = 2, 1024, 1024, 16
D = DM // H            # 64, head dim
HG = 4                 # heads per core
DL = HG * D            # 256, local dm slice
NB = S // 128          # 8 s-blocks
NORM_D = 0.125         # 1/sqrt(D)


def _build_program():
    nc = bacc.Bacc(None, target_bir_lowering=False)

    qT_in = nc.declare_dram_parameter("qT", [DM, S], BF16, isOutput=False)
    kT_in = nc.declare_dram_parameter("kT", [DM, S], BF16, isOutput=False)
    vT_in = nc.declare_dram_parameter("vT", [DM, S], BF16, isOutput=False)
    pT_in = nc.declare_dram_parameter("pT", [DL, S], BF16, isOutput=False)
    wq_in = nc.declare_dram_parameter("wq", [DM, DL], BF16, isOutput=False)
    wk_in = nc.declare_dram_parameter("wk", [DM, DL], BF16, isOutput=False)
    wv_in = nc.declare_dram_parameter("wv", [DM, DL], BF16, isOutput=False)
    wc_in = nc.declare_dram_parameter("wc", [DL, S], BF16, isOutput=False)
    wqb_in = nc.declare_dram_parameter("wqb", [128, 2], F32, isOutput=False)
    wkb_in = nc.declare_dram_parameter("wkb", [128, 2], F32, isOutput=False)
    wvb_in = nc.declare_dram_parameter("wvb", [1, DL], BF16, isOutput=False)
    ones_in = nc.declare_dram_parameter("ones1", [1, 512], BF16, isOutput=False)
    mask_in = nc.declare_dram_parameter("maskLE", [128, 128], BF16, isOutput=False)
    ident_in = nc.declare_dram_parameter("ident", [128, 128], BF16, isOutput=False)
    inv_in = nc.declare_dram_parameter("invidx", [128, NB], F32, isOutput=False)
    out_d = nc.declare_dram_parameter("out", [S, DM], BF16, isOutput=True)

    with TileContext(nc) as tc:
        with tc.tile_pool(name="persist", bufs=1) as cp, \
             tc.tile_pool(name="pm", bufs=4, space="PSUM") as pm, \
             tc.tile_pool(name="scr", bufs=2) as sp:

            maskLE = cp.tile([128, 128], BF16)
            ident = cp.tile([128, 128], BF16)
            invidx = cp.tile([128, NB], F32)
            wqb = cp.tile([128, 2], F32)
            wkb = cp.tile([128, 2], F32)
            wvb = cp.tile([1, DL], BF16)
            ones1 = cp.tile([1, 512], BF16)
            pTt = cp.tile([128, 2, S], BF16)
            qhT = cp.tile([128, 2, S], BF16)
            khT = cp.tile([128, 2, S], BF16)
            vh = cp.tile([128, NB, DL], BF16)
            oT = cp.tile([128, 2, S], BF16)
            wct = cp.tile([128, 2, S], BF16)
            # double-buffered big per-head tensors (fp8 score path)
            a1s = [cp.tile([128, NB, S], FP8, name=f"a1_{x}") for x in range(2)]
            # W'^T, j-pair interleaved: [jp, piece, chunk, t, jlo]
            wtT2s = [cp.tile([128, NB, 4, 256], FP8E5, name=f"wtT2_{x}")
                     for x in range(2)]

            st_sq = {}      # (h,i) -> [128, 2, 128] fp8 strip pair
            st_a1b0 = {}    # h -> bf16 copy of A1 chunk 0
            st_dps = {}
            st_wt0 = {}     # h -> bf16 W'^T chunk 0
            st_wb = {}      # (h,i) -> W' block fp8 (exp, unnormalized)
            st_gsc = {}     # (h,i) -> 1/((t+1) den) column
            st_at = {}      # (h,i) -> A1^T u16-pair strips [128, 4, 128, 2]
            st_dsb = {}     # (h,i) -> masked D_ii^T bf16
            st_nsb = {}     # (h,i) -> N snapshot through chunk i (pair-j order)
            msbs = {}       # (c, parity) -> M snapshot half, bf16 SBUF
            khSs = {}       # h -> kh [s,d] strips bf16
            khS8s = {}      # h -> fp8 copy for DR M updates
            oNs = {}

            def a1_gen(h):
                """A1 = elu(qh ph^T)+1 = min(exp(x),1) + relu(x); 8 units of
                [128,1024] (2 MMs + one exp/min/stt each).  Chunk 0 also keeps
                a bf16 copy (exponent precision at small t)."""
                g, p0 = h // 2, (h % 2) * 64
                a1 = a1s[h % 2]
                a1b0 = sp.tile([128, S], BF16, tag="a1b0", bufs=2, name="a1b0")
                st_a1b0[h] = a1b0
                for m in range(NB):
                    ps = pm.tile([128, 1024], F32, tag="a1ps", bufs=3,
                                 name="ps_a1")
                    for c in range(2):
                        nc.tensor.matmul(
                            ps[:, c * 512:(c + 1) * 512],
                            qhT[p0:p0 + 64, g, m * 128:(m + 1) * 128],
                            pTt[p0:p0 + 64, g, c * 512:(c + 1) * 512],
                            start=True, stop=True)
                    e = sp.tile([128, 1024], BF16, tag="e", bufs=3, name="e")
                    nc.scalar.activation(e[:], ps[:], ACTF.Exp)
                    e1 = sp.tile([128, 1024], BF16, tag="e1", bufs=3, name="e1")
                    nc.gpsimd.tensor_scalar_min(e1[:], e[:], 1.0)
                    if m == 0:
                        nc.vector.scalar_tensor_tensor(
                            a1b0[:], ps[:], 0.0, e1[:], ALU.max, ALU.add)
                        nc.gpsimd.tensor_copy(a1[:, 0, :], a1b0[:])
                    else:
                        nc.vector.scalar_tensor_tensor(
                            a1[:, m, :], ps[:], 0.0, e1[:], ALU.max, ALU.add)
                    yield

            gens = {}

            def pull(h, n):
                if h < HG:
                    if h not in gens:
                        gens[h] = a1_gen(h)
                    for _ in range(n):
                        if next(gens[h], "done") == "done":
                            break

            # ---------------- projections ----------------
            vp_cm = tc.tile_pool(name="vproj", bufs=1)
            vp = vp_cm.__enter__()
            wvt = vp.tile([128, NB, DL], BF16)
            vTt = vp.tile([128, NB, S], BF16)
            with tc.tile_pool(name="proj", bufs=1) as jp:
                wqt = jp.tile([128, NB, DL], BF16)
                wkt = jp.tile([128, NB, DL], BF16)
                qTt = jp.tile([128, NB, S], BF16)
                kTt = jp.tile([128, NB, S], BF16)
                for wt_, wsrc, xt_, xsrc in ((wqt, wq_in, qTt, qT_in),
                                             (wkt, wk_in, kTt, kT_in),
                                             (wvt, wv_in, vTt, vT_in)):
                    for q4 in range(4):
                        kb = 2 * q4
                        nc.sync.dma_start(
                            out=wt_[:, kb:kb + 2, :],
                            in_=wsrc[kb * 128:(kb + 2) * 128, :].rearrange(
                                "(a p) d -> p a d", p=128))
                        nc.sync.dma_start(
                            out=xt_[:, kb:kb + 2, :],
                            in_=xsrc[kb * 128:(kb + 2) * 128, :].rearrange(
                                "(a p) t -> p a t", p=128))
                    if wt_ is wqt:
                        nc.sync.dma_start(
                            out=pTt[:], in_=pT_in.rearrange("(g p) t -> p g t", p=128))
                        nc.sync.dma_start(out=wqb[:], in_=wqb_in[:])
                        nc.sync.dma_start(out=invidx[:], in_=inv_in[:])
                    elif wt_ is wkt:
                        nc.sync.dma_start(out=maskLE[:], in_=mask_in[:])
                        nc.sync.dma_start(out=ident[:], in_=ident_in[:])
                        nc.sync.dma_start(out=wkb[:], in_=wkb_in[:])
                    else:
                        nc.sync.dma_start(out=wvb[:], in_=wvb_in[:])
                        nc.sync.dma_start(out=ones1[:], in_=ones_in[:])
                        nc.sync.dma_start(
                            out=wct[:], in_=wc_in.rearrange("(a p) t -> p a t", p=128))

                # qhT[dm, t] = sum_c wq[c, dm] qT[c, t]  (+bias, * 1/sqrt(D))
                for wt_, xt_, dst, bias_t, scale in (
                    (wqt, qTt, qhT, wqb, NORM_D),
                    (wkt, kTt, khT, wkb, 1.0),
                ):
                    for g in range(2):
                        for n in range(2):
                            ps = pm.tile([128, 512], F32, tag="mm", name="ps_proj")
                            for kb in range(NB):
                                nc.tensor.matmul(
                                    ps[:], wt_[:, kb, g * 128:(g + 1) * 128],
                                    xt_[:, kb, n * 512:(n + 1) * 512],
                                    start=(kb == 0), stop=(kb == NB - 1))
                            nc.scalar.activation(
                                dst[:, g, n * 512:(n + 1) * 512], ps[:],
                                ACTF.Identity, bias=bias_t[:, g:g + 1], scale=scale)
                        if wt_ is wqt and g == 0:
                            # head 0 (g=0) A1 units can start as soon as the
                            # g0 q-projection lands; overlaps the k projection
                            pull(0, 16)

            # ---------------- attention (4 heads, chunked) ----------------
            def emit_khS(h):
                g, p0 = h // 2, (h % 2) * 64
                khS = sp.tile([128, NB - 2, 64], BF16, tag="khS", bufs=3,
                              name="khS")
                nc.sync.dma_start_transpose(
                    out=khS[:], in_=khT[p0:p0 + 64, g, 0:(NB - 2) * 128])
                khSs[h] = khS
                khS8 = sp.tile([128, NB - 2, 64], FP8, tag="khS8", bufs=3,
                               name="khS8")
                nc.gpsimd.tensor_copy(khS8[:], khS[:])
                khS8s[h] = khS8

            def emit_sq(h, i):
                # strip pair [128 s, 2, 128 t] fp8: slot1 = diag (masked),
                # slot0 = lagged unmasked strip (odd i only)
                g, p0 = h // 2, (h % 2) * 64
                if i == 0:
                    sq = sp.tile([128, 1, 128], BF16, tag="sq0", bufs=2,
                                 name="sq0")
                else:
                    sq = sp.tile([128, 2, 128], FP8, tag="sq", bufs=4,
                                 name="sq")
                for si in ([i - 1, i] if i % 2 == 1 else [i]):
                    ps = pm.tile([128, 128], F32, tag="mm", name="ps_sq")
                    nc.tensor.matmul(
                        ps[:], khT[p0:p0 + 64, g, si * 128:(si + 1) * 128],
                        qhT[p0:p0 + 64, g, i * 128:(i + 1) * 128],
                        start=True, stop=True)
                    if si == i:
                        nc.vector.tensor_tensor(sq[:, 1 if i else 0, :], ps[:],
                                                maskLE[:], ALU.mult)
                    else:
                        nc.vector.tensor_copy(sq[:, 0, :], ps[:])
                st_sq[(h, i)] = sq

            def emit_u_half(h, i, c):
                # U(i) = SqT strips @ A1 + qh_i @ M_{<i};  W' = exp(U/(t+1))
                g, p0 = h // 2, (h % 2) * 64
                a1 = a1s[h % 2]
                if c == 0:
                    if i == 0:
                        wb = sp.tile([128, S], BF16, tag="wblk0", bufs=2,
                                     name="wb0")
                    else:
                        wb = sp.tile([128, S], FP8E5, tag="wblk", bufs=4,
                                     name="wb")
                    st_wb[(h, i)] = wb
                    st_dps[(h, i)] = []
                wb = st_wb[(h, i)]
                sq = st_sq[(h, i)]
                mlag = 2 * (i // 2) - 1   # M snapshot (odd index) U(i) reads
                dps = st_dps[(h, i)]
                if True:
                    ps = pm.tile([128, 512], F32, tag="mm", name="ps_u")[:]
                    if i == 0:
                        # full-bf16 chunk 0: |U/(t+1)| is large only here, so
                        # fp8 noise in exponents/weights is confined to i>0
                        nc.tensor.matmul(ps, sq[:, 0, :],
                                         st_a1b0[h][:, c * 512:(c + 1) * 512],
                                         start=True, stop=True)
                    elif i % 2 == 1:
                        nc.tensor.matmul(ps, sq[:],
                                         a1[:, i - 1:i + 1,
                                            c * 512:(c + 1) * 512],
                                         start=True, stop=(mlag < 0),
                                         perf_mode=DRMODE)
                    else:
                        nc.tensor.matmul(ps, sq[:, 1, :],
                                         a1[:, i, c * 512:(c + 1) * 512],
                                         start=True, stop=(mlag < 0))
                    if mlag >= 0:
                        nc.tensor.matmul(
                            ps, qhT[p0:p0 + 64, g, i * 128:(i + 1) * 128],
                            msbs[(c, (mlag // 2) % 2)][p0:p0 + 64, :],
                            start=False, stop=True)
                    dp = sp.tile([128, 1], F32, tag="dp", bufs=6,
                                 name="dp")
                    nc.scalar.activation(
                        wb[:, c * 512:(c + 1) * 512], ps, ACTF.Exp,
                        scale=invidx[:, i:i + 1], accum_out=dp[:])
                    dps.append(dp)
                if c == 0:
                    return

            def emit_u_rest(h, i):
                g, p0 = h // 2, (h % 2) * 64
                a1 = a1s[h % 2]
                sq = st_sq.pop((h, i))
                dps = st_dps.pop((h, i))
                dsum = sp.tile([128, 1], F32, tag="dsum", bufs=2,
                               name="dsum")
                nc.vector.tensor_tensor(dsum[:], dps[0][:], dps[1][:], ALU.add)
                rec = sp.tile([128, 1], F32, tag="rec", bufs=2, name="rec")
                nc.vector.reciprocal(rec[:], dsum[:])
                gsc = sp.tile([128, 1], F32, tag="gsc", bufs=8, name="gsc")
                nc.vector.tensor_tensor(gsc[:], rec[:], invidx[:, i:i + 1],
                                        ALU.mult)
                st_gsc[(h, i)] = gsc
                # M delta for chunks (i-1, i) at odd boundaries; chunks >=
                # NB-2 are past the last snapshot and never read
                if i % 2 == 1 and i < NB - 1:
                    dmb = pm.tile([64, 1024], F32, tag="a1ps", bufs=3,
                                  name="ps_mdel")
                    for c in range(2):
                        dm = dmb[:, c * 512:(c + 1) * 512]
                        nc.tensor.matmul(
                            dm, khS8s[h][:, i - 1:i + 1, :],
                            a1[:, i - 1:i + 1, c * 512:(c + 1) * 512],
                            start=True, stop=True, perf_mode=DRMODE)
                        msb = sp.tile([128, 512], BF16, tag=f"msb{c}", bufs=3,
                                      name="msb")
                        if i == 1:
                            nc.vector.tensor_copy(msb[p0:p0 + 64, :], dm)
                        else:
                            nc.vector.tensor_tensor(
                                msb[p0:p0 + 64, :], dm,
                                msbs[(c, ((i - 2) // 2) % 2)][p0:p0 + 64, :],
                                ALU.add)
                        msbs[(c, (i // 2) % 2)] = msb


            def emit_wt(h, i):
                # W'^T via one u16-pair-viewed crossbar transpose; the 512
                # logical u16 rows (j-pairs) wrap into [jp%128, piece, t].
                # Chunk 0 transposes plain bf16 instead (only emit_d(0) reads)
                wb = st_wb.pop((h, i))
                if i == 0:
                    wt0 = sp.tile([128, NB, 128], BF16, tag="wtT0", bufs=2,
                                  name="wtT0")
                    nc.sync.dma_start_transpose(out=wt0[:], in_=wb[:])
                    st_wt0[h] = wt0
                    return
                wt2 = wtT2s[h % 2]
                nc.sync.dma_start_transpose(
                    out=wt2[:, i, :, :].bitcast(U16),
                    in_=wb[:].bitcast(U16))

            def emit_at(h, i):
                if i == 0:
                    at0 = sp.tile([128, NB, 128], BF16, tag="at0", bufs=2,
                                  name="at0")
                    nc.sync.dma_start_transpose(out=at0[:],
                                                in_=st_a1b0[h][:])
                    st_at[(h, i)] = at0
                    return
                a1 = a1s[h % 2]
                at = sp.tile([128, 4, 256], FP8, tag="a1T", bufs=5,
                             name="a1T")
                nc.sync.dma_start_transpose(
                    out=at[:].bitcast(U16),
                    in_=a1[:, i, :].bitcast(U16))
                st_at[(h, i)] = at

            def emit_d(h, i):
                # D_ii^T[s',t'] = sum_j A1[s,j] W'[t,j], masked to s<=t;
                # 4 j-pieces x even/odd stride-2 slices
                at = st_at.pop((h, i))
                wt2 = wtT2s[h % 2]
                ps = pm.tile([128, 128], F32, tag="mm", name="ps_d")
                if i == 0:
                    wt0 = st_wt0.pop(h)
                    for kb in range(NB):
                        nc.tensor.matmul(ps[:], at[:, kb, :], wt0[:, kb, :],
                                         start=(kb == 0), stop=(kb == NB - 1))
                    dsb = sp.tile([128, 128], BF16, tag="dsb", bufs=4,
                                  name="dsb")
                    nc.vector.tensor_tensor(dsb[:], ps[:], maskLE[:], ALU.mult)
                    st_dsb[(h, i)] = dsb
                    return
                for p in range(4):
                    atp = at[:, p, :].rearrange("q (t k) -> q t k", k=2)
                    wtp = wt2[:, i, p, :].rearrange("q (t k) -> q t k", k=2)
                    for eo in range(2):
                        nc.tensor.matmul(
                            ps[:], atp[:, :, eo], wtp[:, :, eo],
                            start=(p == 0 and eo == 0),
                            stop=(p == 3 and eo == 1))
                dsb = sp.tile([128, 128], BF16, tag="dsb", bufs=4, name="dsb")
                nc.vector.tensor_tensor(dsb[:], ps[:], maskLE[:], ALU.mult)
                st_dsb[(h, i)] = dsb

            def emit_nupd(h, i):
                # N_i[j',d] = N_{i-1} + A1_i^T vh_i in pair-j order
                # j' = (piece, eo, jp); mixed fp8 lhsT x bf16 rhs
                if i == NB - 1:
                    return
                a1 = a1s[h % 2]
                d0 = h * 64
                ps = pm.tile([128, 8, 64], F32, tag="mm", name="ps_n")
                for p in range(4):
                    for eo in range(2):
                        nc.tensor.matmul(
                            ps[:, 2 * p + eo, :],
                            a1[:, i, 256 * p:256 * (p + 1)].rearrange(
                                "q (s k) -> q s k", k=2)[:, :, eo],
                            vh[:, i, d0:d0 + 64], start=True, stop=True)
                nsb = sp.tile([128, 8, 64], BF16, tag="nsb", bufs=5, name="nsb")
                if i == 0:
                    nc.vector.tensor_copy(nsb[:], ps[:])
                else:
                    nc.vector.tensor_tensor(nsb[:], ps[:],
                                            st_nsb[(h, i - 1)][:], ALU.add)
                st_nsb[(h, i)] = nsb

            def emit_o2(h, i):
                # out2(i) = (W'_i @ N_{<i} + D^T-contract vh_i) * gsc
                d0 = h * 64
                wt2 = wtT2s[h % 2]
                if h % 2 == 0 and i == 0:
                    oNs[h // 2] = sp.tile([128, NB, 128], BF16, tag="oN",
                                          bufs=2, name="oN")
                oN = oNs[h // 2]
                ps = pm.tile([128, 64], F32, tag="mm", name="ps_o2")
                dsb = st_dsb.pop((h, i))
                if i > 0:
                    nsb = st_nsb[(h, i - 1)]
                    for p in range(4):
                        wtp = wt2[:, i, p, :].rearrange("q (t k) -> q t k", k=2)
                        for eo in range(2):
                            nc.tensor.matmul(
                                ps[:], wtp[:, :, eo],
                                nsb[:, 2 * p + eo, :],
                                start=(p == 0 and eo == 0), stop=False)
                    nc.tensor.matmul(ps[:], dsb[:], vh[:, i, d0:d0 + 64],
                                     start=False, stop=True)
                else:
                    nc.tensor.matmul(ps[:], dsb[:], vh[:, i, d0:d0 + 64],
                                     start=True, stop=True)
                if i >= 2:
                    st_nsb.pop((h, i - 2), None)
                nc.scalar.activation(
                    oN[:, i, (h % 2) * 64:(h % 2) * 64 + 64], ps[:],
                    ACTF.Copy, scale=st_gsc.pop((h, i))[:])
                if h == 1 and i == NB - 1:
                    nc.sync.dma_start_transpose(
                        out=oT[:, 0, :].rearrange("p (a b) -> p a b", b=128),
                        in_=oN[:].rearrange("p a b -> p (a b)"))
                elif h == HG - 1:
                    if i == NB - 1:
                        tpo = pm.tile([128, 128], BF16, tag="mm", name="tpo")
                        nc.tensor.transpose(tpo[:], oN[:, i, :], ident[:])
                        nc.scalar.activation(
                            oT[:, h // 2, i * 128:(i + 1) * 128], tpo[:],
                            ACTF.Copy)
                    else:
                        nc.sync.dma_start_transpose(
                            out=oT[:, 1, i * 128:(i + 1) * 128],
                            in_=oN[:, i, :])

            def emit_final_tile(i):
                # i covers row-blocks 2i, 2i+1; one DMA per 256 output rows
                ot = sp.tile([128, 2, S], BF16, tag="ot", bufs=2, name="ot")
                for z in range(2):
                    ib = 2 * i + z
                    for c in range(2):
                        ps = pm.tile([128, 512], F32, tag="mm", name="ps_fin")
                        for g2 in range(2):
                            nc.tensor.matmul(
                                ps[:], oT[:, g2, ib * 128:(ib + 1) * 128],
                                wct[:, g2, c * 512:(c + 1) * 512],
                                start=(g2 == 0), stop=(g2 == 1))
                        if (ib + c) % 2 == 0:
                            nc.scalar.activation(
                                ot[:, z, c * 512:(c + 1) * 512], ps[:], ACTF.Copy)
                        else:
                            nc.vector.tensor_copy(
                                ot[:, z, c * 512:(c + 1) * 512], ps[:])
                nc.sync.dma_start(
                    out=out_d[2 * i * 128:(2 * i + 2) * 128, :].rearrange(
                        "(a p) d -> p a d", p=128),
                    in_=ot[:])

            # vh[s, d] = sum_c vT[c, s] wv[c, d] + wv_b[d]

            def emit_vh(m):
                ps = pm.tile([128, DL], F32, tag="mm", name="ps_vh")
                for kb in range(NB):
                    nc.tensor.matmul(
                        ps[:], vTt[:, kb, m * 128:(m + 1) * 128], wvt[:, kb, :],
                        start=(kb == 0), stop=False)
                nc.tensor.matmul(ps[:], ones1[:, 0:128], wvb[:], start=False, stop=True)
                nc.scalar.activation(vh[:, m, :], ps[:], ACTF.Copy)

            pull(0, 32)  # no-op remainder if already pulled

            def hi(tau):
                # map absolute pipeline time to (head, iter), None past the end
                h, i = divmod(tau, NB)
                return (h, i) if 0 <= h < HG else None

            emit_khS(0)
            emit_sq(0, 0)
            for tau in range(HG * NB + 4):
                cur = hi(tau)
                if tau <= NB - 1:
                    emit_vh(tau)
                for hh in range(HG):
                    ii = tau - NB * hh - 3
                    if 0 <= ii <= NB - 1:
                        emit_o2(hh, ii)
                if cur:
                    nxt = hi(tau + 1)
                    if nxt:
                        if nxt[1] == 0:
                            emit_khS(nxt[0])
                        emit_sq(*nxt)
                    emit_u_half(*cur, 0)
                for hh in range(HG):
                    ii = tau - NB * hh - 2
                    if 0 <= ii <= NB - 1:
                        emit_d(hh, ii)
                if cur:
                    emit_u_half(*cur, 1)
                    emit_u_rest(*cur)
                    emit_wt(*cur)
                pull(tau // NB + 1, 1)
                if cur:
                    emit_at(*cur)
                if hi(tau - 2):
                    emit_nupd(*hi(tau - 2))
                pull(tau // NB + 1, 1)
                ft = tau - (3 * NB + 5)
                if ft >= 0 and ft % 2 == 0:
                    emit_final_tile(ft // 2)
            st_nsb.clear()
            oNs.clear()

            vp_cm.__exit__(None, None, None)

    nc.finalize()
    return nc


_CACHE = {}


def _get_program():
    if "nc" not in _CACHE:
        _CACHE["nc"] = _build_program()
    return _CACHE["nc"]


def _consts():
    if "consts" not in _CACHE:
        p_ = np.arange(128, dtype=np.float32)[:, None]
        c_ = np.arange(128, dtype=np.float32)[None, :]
        maskLE = (p_ <= c_).astype(NPBF)
        ident = np.eye(128, dtype=np.float32).astype(NPBF)
        blk = np.arange(NB, dtype=np.float32)[None, :]
        invidx = (1.0 / (blk * 128.0 + p_ + 1.0)).astype(np.float32)
        ones1 = np.ones((1, 512), NPBF)
        _CACHE["consts"] = (maskLE, ident, invidx, ones1)
    return _CACHE["consts"]


PROFILE = False
LAST_RESULTS = None


def kernel(v, k, q, p, wq_k, wq_b, wk_k, wk_b, wv_k, wv_b, wc_k, wc_b):
    global LAST_RESULTS
    nc = _get_program()
    maskLE, ident, invidx, ones1 = _consts()

    qT = [np.ascontiguousarray(q[b].T).astype(NPBF) for b in range(B)]
    kT = [np.ascontiguousarray(k[b].T).astype(NPBF) for b in range(B)]
    vT = [np.ascontiguousarray(v[b].T).astype(NPBF) for b in range(B)]
    pT = [np.ascontiguousarray(p[b].T).astype(NPBF) for b in range(B)]
    wqc = wq_k.astype(NPBF)
    wkc = wk_k.astype(NPBF)
    wvc = wv_k.astype(NPBF)
    wcc = wc_k.astype(NPBF)

    in_maps = []
    for c in range(8):
        b, hg = c // 4, c % 4
        c0 = hg * DL
        wqb = np.ascontiguousarray(
            (wq_b[c0:c0 + DL].reshape(2, 128).T * NORM_D).astype(np.float32))
        wkb = np.ascontiguousarray(wk_b[c0:c0 + DL].reshape(2, 128).T.astype(np.float32))
        in_maps.append({
            "qT": qT[b], "kT": kT[b], "vT": vT[b],
            "pT": np.ascontiguousarray(pT[b][c0:c0 + DL]),
            "wq": np.ascontiguousarray(wqc[:, c0:c0 + DL]),
            "wk": np.ascontiguousarray(wkc[:, c0:c0 + DL]),
            "wv": np.ascontiguousarray(wvc[:, c0:c0 + DL]),
            "wc": np.ascontiguousarray(wcc[c0:c0 + DL, :]),
            "wqb": wqb, "wkb": wkb,
            "wvb": np.ascontiguousarray(wv_b[c0:c0 + DL].reshape(1, DL).astype(NPBF)),
            "ones1": ones1, "maskLE": maskLE, "ident": ident,
            "invidx": invidx,
        })

    res = run_bass_kernel_spmd(
        nc, in_maps, core_ids=list(range(8)), trace=PROFILE)
    LAST_RESULTS = res

    out = np.zeros((B, S, DM), np.float32)
    for c in range(8):
        out[c // 4] += res.results[c]["out"].astype(np.float32)
    out += wc_b[None, None, :].astype(np.float32)
    return out
